# revision 38
# baseline (speedup 1.0000x reference)
"""Trainium2 Bass kernel for 3D multi-head attention (nn_Attention3D).

Problem: x [1, 16, 16, 16, 528] -> full attention over N=4096 tokens,
8 heads of dim 66, qkv + out projections.

Sharding: one head per NeuronCore (8 cores). Each core computes its
head's q/k/v projections, full 4096x4096 attention, and its partial
contribution to the output projection. Host divides each core's
partial by its softmax denominator (carried out as an extra output
column), sums the 8 partials and adds the output bias.

The kernel is one fused pipeline, scheduled for the PE being the
critical engine (~173us of bf16 matmul streaming vs ~140us of ScalarE
exp; DVE/DMA far below):

  - x loads in 10 wide DMAs (dma_start issue costs ~700ns serial on
    SyncE, so few big transfers beat many small ones); a few dummy
    matmuls walk the PE up its p-state ramp while the first half lands.
  - q-block 0's scores+exp run interleaved INTO the q/k projection
    loop (its 4-chunk groups use the sca PSUM pool, which phase A
    leaves room for by running q/k with 2 PSUM bufs), so ScalarE
    starts ~9us in instead of after all projections.
  - blocks 1-7 run 4/3-chunk exp groups double-buffered across two
    PSUM pools (4+3 banks) + 1 bank for the AV accumulator = 8. AV
    matmuls are emitted from a global catch-up FIFO (<=2 per group
    slot) so block 0's deferred AV work replays during block 1 and
    the pipeline re-converges to one-group-behind; the FIFO also
    carries each block's oT cast and out-projection, which therefore
    interleave into the following block with no separate phase.
  - softmax denominators ride along as a ones-column in the v weights
    (PSUM partition 0 of the AV accumulator), get copied into the
    bf16 proj stationary (row 0), and a 1.0 in an extra wp column
    emits them as y[:, 528] through the same projection matmul.

Layout notes (host-side prep, free): x is pre-transposed to
xT [640, 4096] (C on partitions) with row 528 = 1.0 (bias row) and
rows 529-639 zero, so every matmul contraction sits on the partition
dim in K=128 chunks with the qkv biases folded in. q is pre-scaled by
hd^-0.5. All matmul operands are bf16 (PE native 1 col/cycle);
PSUM accumulation is fp32. Measured rel err vs fp32 reference ~2e-3.
"""

import numpy as np

import ml_dtypes

BF16_NP = ml_dtypes.bfloat16

EMBED = 528
EOUT = 536  # proj output cols: 528 data + denom col (528) + pad
HD = 66
NHEADS = 8
NT = 4096
NCH = 5  # contraction chunks of 128 (640 = 528 + bias row + pad)


def _build_nc(nt=NT):
    import concourse.tile as tile
    from concourse import bacc, mybir

    F32 = mybir.dt.float32
    BF16 = mybir.dt.bfloat16
    AF = mybir.ActivationFunctionType

    nkc = nt // 128  # k-token chunks (32)
    nqb = nt // 512  # q-token blocks (8)

    # blocks 1..: group sizes 4,3,4,3,4,3,4,3,2,2 (32 chunks). The two
    # 2-chunk tail groups keep the next block's first sca tile from
    # ring-waiting on an exp ScalarE hasn't reached yet.
    sizes = [4, 3, 4, 3, 4, 3, 4, 3, 2, 2]
    assert sum(sizes) == nkc
    groups = []
    kc0 = 0
    for gsz in sizes:
        groups.append((kc0, gsz))
        kc0 += gsz

    nc = bacc.Bacc("TRN2", target_bir_lowering=False, debug=False)
    xT_d = nc.dram_tensor("xT", [128, NCH, nt], BF16, kind="ExternalInput").ap()
    wq_d = nc.dram_tensor("wq", [128, NCH, 128], BF16, kind="ExternalInput").ap()
    wk_d = nc.dram_tensor("wk", [128, NCH, 128], BF16, kind="ExternalInput").ap()
    wv_d = nc.dram_tensor("wv", [128, NCH, HD + 2], BF16, kind="ExternalInput").ap()
    wp_d = nc.dram_tensor("wp", [128, EOUT], BF16, kind="ExternalInput").ap()
    y_d = nc.dram_tensor("y", [nt, EOUT], BF16, kind="ExternalOutput").ap()

    with tile.TileContext(nc) as tc:
        with (
            tc.tile_pool(name="const", bufs=1) as constp,
            tc.tile_pool(name="persist", bufs=1) as pp,
            tc.tile_pool(name="ep", bufs=16) as ep,
            tc.tile_pool(name="yp", bufs=4) as yp,
            tc.tile_pool(name="psSa", bufs=1, space="PSUM") as psSa,
        ):
            wq = constp.tile([128, NCH, 128], BF16, name="wq_sb")
            wk = constp.tile([128, NCH, 128], BF16, name="wk_sb")
            wv = constp.tile([128, NCH, HD + 2], BF16, name="wv_sb")
            wp = constp.tile([128, EOUT], BF16, name="wp_sb")
            warm = constp.tile([128, 16], BF16, name="warm_sb")

            xT = pp.tile([128, NCH, nt], BF16, name="xT_sb")
            # block 0's x in 5 small per-chunk DMAs first (ready ~5us so
            # qk(0) starts early); blocks 1-7 in 10 wide per-chunk DMAs
            for c in range(NCH):
                nc.sync.dma_start(xT[:, c, 0:512], xT_d[:, c, 0:512])
            nc.sync.dma_start(wq[:], wq_d[:])
            nc.sync.dma_start(wk[:], wk_d[:])
            for c in range(NCH):
                nc.sync.dma_start(xT[:, c, 512:2048], xT_d[:, c, 512:2048])
            for c in range(NCH):
                nc.sync.dma_start(xT[:, c, 2048:nt], xT_d[:, c, 2048:nt])
            nc.sync.dma_start(wv[:], wv_d[:])
            nc.sync.dma_start(wp[:], wp_d[:])

            # qT/kT are hd-padded to 128 partitions (rows HD.. stay 0) so
            # scores contract over a full K=128.
            qT = pp.tile([128, nt], BF16, name="qT")
            kT = pp.tile([128, nt], BF16, name="kT")
            vaug = pp.tile([128, nkc, HD + 2], BF16, name="vaug")
            # out-proj stationary per block, double-buffered; rows 68-127
            # must read zero in the proj matmul, so memset once and only
            # ever write rows 0..67.
            oT = [pp.tile([128, 512], BF16, name=f"oT{i}") for i in range(2)]
            nc.gpsimd.memset(warm[:], 0)
            nc.gpsimd.memset(oT[0][:], 0)
            nc.gpsimd.memset(oT[1][:], 0)

            # ---- deferred-work FIFO: AV groups, oT casts, projections ----
            o_ps_tiles = {}
            avq = []
            slot = [0]  # global group-slot counter for the AV age guard

            def sc_tile_a():
                return psSa.tile([128, 4 * 512], F32, tag="sca", name="sca")

            def pop_work(budget, floor=0):
                spent = 0
                while len(avq) > floor and spent < budget:
                    item = avq[0]
                    kind = item[0]
                    if kind == "av":
                        _, b, E, g0, gsz, pushed = item
                        if b not in o_ps_tiles:
                            o_ps_tiles[b] = psO.tile(
                                [HD + 2, 512], F32, tag="o", name="o_ps"
                            )
                        o_ps = o_ps_tiles[b]
                        for j in range(gsz):
                            kc = g0 + j
                            nc.tensor.matmul(
                                o_ps[:],
                                vaug[:, kc, :],
                                E[:, j * 512 : (j + 1) * 512],
                                start=(kc == 0),
                                stop=(kc == nkc - 1),
                                skip_group_check=True,
                            )
                        spent += 1
                    elif kind == "cast":
                        b = item[1]
                        nc.vector.tensor_copy(
                            oT[b % 2][0 : HD + 2, :], o_ps_tiles[b][:]
                        )
                    else:  # proj piece: one 128-token slice of block b
                        _, b, t = item
                        oTt = oT[b % 2]
                        pool = psSb if t < 2 else psSa
                        tag = "scb" if t < 2 else "sca"
                        pt = pool.tile([128, EOUT], F32, tag=tag, name="pt")
                        r0 = b * 512 + t * 128
                        st = oTt[:, t * 128 : (t + 1) * 128]
                        nc.tensor.matmul(
                            pt[:, :512], st, wp[:, :512], start=True, stop=True
                        )
                        nc.tensor.matmul(
                            pt[:, 512:EOUT],
                            st,
                            wp[:, 512:EOUT],
                            start=True,
                            stop=True,
                        )
                        ysb = yp.tile([128, EOUT], BF16, tag="ysb", name="ysb")
                        nc.vector.tensor_copy(ysb[:], pt[:])
                        # two row-half DMAs land on two queues so the last
                        # writes drain in parallel
                        nc.sync.dma_start(y_d[r0 : r0 + 64, :], ysb[0:64, :])
                        nc.sync.dma_start(
                            y_d[r0 + 64 : r0 + 128, :], ysb[64:128, :]
                        )
                        spent += 1
                    avq.pop(0)

            def push_block_done(b):
                avq.append(("cast", b))
                for t in range(4):
                    avq.append(("proj", b, t))

            def emit_group(b, g0, gsz, sc):
                qs = slice(b * 512, (b + 1) * 512)
                for j in range(gsz):
                    kc = g0 + j
                    nc.tensor.matmul(
                        sc[:, j * 512 : (j + 1) * 512],
                        kT[:, kc * 128 : (kc + 1) * 128],
                        qT[:, qs],
                        start=True,
                        stop=True,
                    )
                E = ep.tile([128, 4 * 512], BF16, tag="E", name="E")
                nc.scalar.activation(E[:, : gsz * 512], sc[:, : gsz * 512], AF.Exp)
                avq.append(("av", b, E, g0, gsz, slot[0]))
                slot[0] += 1

            # ---------------- Phase A + block-0 scores ----------------
            with tc.tile_pool(name="psA", bufs=1, space="PSUM") as psA:
                # PE p-state warmup through the sca ring while x DMA lands
                wps = sc_tile_a()
                for _ in range(60):
                    nc.tensor.matmul(
                        wps[0:16, 0:16], warm[:], warm[:], start=True, stop=True
                    )

                for b in range(nqb):
                    qs = slice(b * 512, (b + 1) * 512)
                    ps_q = psA.tile([128, 512], F32, tag="qk", bufs=2, name="ps_q")
                    ps_k = psA.tile([128, 512], F32, tag="qk", bufs=2, name="ps_k")
                    for c in range(NCH):
                        for w, ps in ((wq, ps_q), (wk, ps_k)):
                            nc.tensor.matmul(
                                ps[:],
                                w[:, c, :],
                                xT[:, c, qs],
                                start=(c == 0),
                                stop=(c == NCH - 1),
                            )
                    nc.vector.tensor_copy(qT[:, qs], ps_q[:])
                    nc.vector.tensor_copy(kT[:, qs], ps_k[:])
                    # block 0's scores group over the k-chunks this qk block
                    # just produced; exp starts ~9us into the kernel
                    emit_group(0, 4 * b, 4, sc_tile_a())
                # v: two token-block chains in flight
                for t0 in range(0, nkc, 2):
                    psvs = [
                        psA.tile([128, HD + 2], F32, tag="v", bufs=2, name="ps_v")
                        for _ in range(2)
                    ]
                    for c in range(NCH):
                        for i in range(2):
                            ts_ = slice((t0 + i) * 128, (t0 + i + 1) * 128)
                            nc.tensor.matmul(
                                psvs[i][:],
                                xT[:, c, ts_],
                                wv[:, c, :],
                                start=(c == 0),
                                stop=(c == NCH - 1),
                            )
                    for i in range(2):
                        nc.vector.tensor_copy(vaug[:, t0 + i, :], psvs[i][:])
            push_block_done(0)

            # ------------- Phase B: blocks 1-7 + deferred work -------------
            with (
                tc.tile_pool(name="psSb", bufs=1, space="PSUM") as psSb,
                tc.tile_pool(name="psO", bufs=1, space="PSUM") as psO,
            ):
                for b in range(1, nqb):
                    for gi, (g0, gsz) in enumerate(groups):
                        if gsz == 4:
                            sc = sc_tile_a()
                        else:
                            sc = psSb.tile(
                                [128, 3 * 512], F32, tag="scb", name="scb"
                            )
                        emit_group(b, g0, gsz, sc)
                        pop_work(2, floor=2)
                    push_block_done(b)
                # drain
                pop_work(10**9)

    nc.compile()
    return nc


def _prep_inputs(x, w_qkv, b_qkv, w_proj, nt):
    """Host-side shard prep: returns list of 8 in_maps."""
    x = np.asarray(x, dtype=np.float32)
    w_qkv = np.asarray(w_qkv, dtype=np.float32)
    b_qkv = np.asarray(b_qkv, dtype=np.float32)
    w_proj = np.asarray(w_proj, dtype=np.float32)

    xt = x.reshape(nt, EMBED)
    xT_pad = np.zeros((NCH * 128, nt), dtype=np.float32)
    xT_pad[:EMBED] = xt.T
    xT_pad[EMBED] = 1.0
    # [128, NCH, nt]: partition-major to match the SBUF tile layout
    xT_in = np.ascontiguousarray(
        xT_pad.reshape(NCH, 128, nt).transpose(1, 0, 2)
    ).astype(BF16_NP)

    s = float(HD) ** -0.5
    in_maps = []
    for h in range(NHEADS):
        sl_q = slice(h * HD, (h + 1) * HD)
        sl_k = slice(EMBED + h * HD, EMBED + (h + 1) * HD)
        sl_v = slice(2 * EMBED + h * HD, 2 * EMBED + (h + 1) * HD)

        wq_t = np.zeros((NCH * 128, 128), dtype=np.float32)
        wq_t[:EMBED, :HD] = (w_qkv[sl_q] * s).T
        wq_t[EMBED, :HD] = b_qkv[sl_q] * s

        wk_t = np.zeros((NCH * 128, 128), dtype=np.float32)
        wk_t[:EMBED, :HD] = w_qkv[sl_k].T
        wk_t[EMBED, :HD] = b_qkv[sl_k]

        # ones column at index 0 so the softmax denominator lands on
        # PSUM partition 0 (-> oT row 0)
        wv_t = np.zeros((NCH * 128, HD + 2), dtype=np.float32)
        wv_t[:EMBED, 1 : HD + 1] = w_qkv[sl_v].T
        wv_t[EMBED, 1 : HD + 1] = b_qkv[sl_v]
        wv_t[EMBED, 0] = 1.0

        # proj weights: row 0 = denom row: zero into data cols, 1.0 into
        # col 528 so y[:, 528] = softmax denominator per token
        wp_t = np.zeros((128, EOUT), dtype=np.float32)
        wp_t[1 : HD + 1, :EMBED] = w_proj[:, sl_q].T
        wp_t[0, EMBED] = 1.0

        in_maps.append(
            {
                "xT": xT_in,
                "wq": np.ascontiguousarray(
                    wq_t.reshape(NCH, 128, 128).transpose(1, 0, 2)
                ).astype(BF16_NP),
                "wk": np.ascontiguousarray(
                    wk_t.reshape(NCH, 128, 128).transpose(1, 0, 2)
                ).astype(BF16_NP),
                "wv": np.ascontiguousarray(
                    wv_t.reshape(NCH, 128, HD + 2).transpose(1, 0, 2)
                ).astype(BF16_NP),
                "wp": wp_t.astype(BF16_NP),
            }
        )
    return in_maps


_NC_CACHE = {}


def _get_nc(nt=NT):
    if nt not in _NC_CACHE:
        _NC_CACHE[nt] = _build_nc(nt)
    return _NC_CACHE[nt]


def kernel(x, w_qkv, b_qkv, w_proj, b_proj, _trace=False):
    from concourse.bass_utils import run_bass_kernel_spmd

    x = np.asarray(x, dtype=np.float32)
    b_proj = np.asarray(b_proj, dtype=np.float32)
    B, D, H, W, C = x.shape
    nt = D * H * W

    nc = _get_nc(nt)
    in_maps = _prep_inputs(x, w_qkv, b_qkv, w_proj, nt)
    res = run_bass_kernel_spmd(
        nc, in_maps, core_ids=list(range(NHEADS)), trace=_trace
    )
    out = np.zeros((nt, EMBED), dtype=np.float32)
    for r in res.results:
        yfull = np.asarray(r["y"], dtype=np.float32)
        out += yfull[:, :EMBED] / yfull[:, EMBED : EMBED + 1]
    out += b_proj
    kernel.last_results = res
    return out.reshape(B, D, H, W, C)
